# revision 2
# baseline (speedup 1.0000x reference)
"""Cross-attention kernel for Trainium2, sharded across 8 NeuronCores.

Sharding: data-parallel over batch (B=2) x tensor-parallel over head groups
(16 heads -> 4 groups of 4). Core c handles batch c//4, head group c%4.
Each core projects with its 256-wide column shard of Wq/Wk/Wv, runs attention
for its 4 heads, applies its 256-row shard of Wo, and a ReduceScatter over
each batch group of 4 cores sums the partial outputs and hands each core its
512-row slice of the final output.

v2 layout: query-half-outer attention loop; the output projection and the
(bf16) ReduceScatter chunk for a half are issued as soon as that half's
attention finishes, so the collective overlaps the other half's compute.
PSUM attention accumulators are drained to SBUF by the (otherwise idle)
Pool engine so softmax normalization never blocks the tensor engine.

Matmul-path data is bf16 (converted host-side); softmax statistics and PSUM
accumulation stay fp32. The attention scale 1/sqrt(64) is folded into Wq on
the host.
"""

import sys

sys.path.insert(0, "/opt/trn_rl_repo")

import numpy as np

import concourse.bass as bass
import concourse.mybir as mybir
import concourse.tile as tile
from concourse import bacc
from concourse.masks import make_identity

F32 = mybir.dt.float32
BF16 = mybir.dt.bfloat16

B = 2
S = 2048  # both Sq and Sk
D = 1024
NCORES = 8
HEADS_PER_CORE = 4
DH = 64
DG = HEADS_PER_CORE * DH  # 256: per-core projection width
TCOL = 512  # token column width for projections / attention moving dim
NTCOL = S // TCOL  # 4
NIC = D // 128  # 8 input-dim chunks
NKB = S // 128  # 16 key blocks
NQB = S // 128  # 16 query blocks
OUT_ROWS = S // 4  # 512 rows of final output per core (ReduceScatter shard)


def build_compute(tc, ins, st, phases=7):
    """One full pass: projections, attention, output projection, and the
    chunked ReduceScatter. May sit inside a timing repeat loop (all state
    tiles are rewritten every pass).

    Bits 0/1/2 enable phases 1/2/3. Timing-only modifier bits (results
    become wrong): bit3 (8) = phase 3 skips the DRAM write; bit4 (16) =
    phase 2 skips the denominator broadcast round-trip."""
    nc = tc.nc
    q, k, v = ins["q"], ins["k"], ins["v"]
    wq_sb, wk_sb, wv_sb, wo_sb = st["wq_sb"], st["wk_sb"], st["wv_sb"], st["wo_sb"]
    bq_sb, bk_sb = st["bq_sb"], st["bk_sb"]
    QT, KT, VA, OTs = st["QT"], st["KT"], st["VA"], st["OTs"]
    partial, dram2 = st["partial"], st["dram2"]
    out_ext = st["out_ext"]

    # ---- Phase 1: PE-transpose inputs + projections ----
    ident = st["ident"]
    if phases & 1:
        with (
            tc.tile_pool(name="nat", bufs=3) as natp,
            tc.tile_pool(name="tT", bufs=3) as tTp,
            tc.tile_pool(name="ps_t", bufs=2, space="PSUM") as ps_t,
            tc.tile_pool(name="ps_p", bufs=4, space="PSUM") as ps_p,
        ):
            for which, src in (("q", q), ("k", k), ("v", v)):
                for tcol in range(NTCOL):
                    nat = natp.tile([128, 4, D], BF16)
                    rows = src[tcol * TCOL : (tcol + 1) * TCOL, :]
                    nc.sync.dma_start(
                        out=nat[:], in_=rows.rearrange("(b p) d -> p b d", p=128)
                    )
                    tT = tTp.tile([128, NIC, TCOL], BF16)
                    for ic in range(NIC):
                        pst = ps_t.tile([128, TCOL], BF16)
                        for tb in range(4):
                            nc.tensor.transpose(
                                pst[:, tb * 128 : (tb + 1) * 128],
                                nat[:, tb, ic * 128 : (ic + 1) * 128],
                                ident[:],
                            )
                        nc.vector.tensor_copy(tT[:, ic, :], pst[:])
                    if which in ("q", "k"):
                        dstT = QT if which == "q" else KT
                        bias = bq_sb if which == "q" else bk_sb
                        w_sb = wq_sb if which == "q" else wk_sb
                        for db in range(2):
                            pp = ps_p.tile([128, TCOL], F32)
                            for ic in range(NIC):
                                nc.tensor.matmul(
                                    pp[:],
                                    w_sb[:, ic, db * 128 : (db + 1) * 128],
                                    tT[:, ic, :],
                                    start=(ic == 0),
                                    stop=(ic == NIC - 1),
                                )
                            nc.vector.tensor_scalar_add(
                                dstT[:, db, tcol * TCOL : (tcol + 1) * TCOL],
                                pp[:],
                                bias[:, db, :],
                            )
                    else:
                        for tb in range(4):
                            pp = ps_p.tile([128, TCOL], F32)
                            for ic in range(NIC):
                                nc.tensor.matmul(
                                    pp[:, 0:DG],
                                    tT[:, ic, tb * 128 : (tb + 1) * 128],
                                    wv_sb[:, ic, :],
                                    start=(ic == 0),
                                    stop=(ic == NIC - 1),
                                )
                            kb = tcol * 4 + tb
                            nc.vector.tensor_copy(
                                VA[:, kb, :, 0:DH],
                                pp[:, 0:DG].rearrange("p (h d) -> p h d", d=DH),
                            )

    # ---- Phases 2+3 per query half, with the RS chunk issued per half ----
    def phase3_qb(qb, psZ, zp):
        """Output projection for one 128-query block (8 matmuls + drain)."""
        zz = psZ.tile([128, 2, TCOL], F32, name=f"zz{qb}", tag="zz")
        for h4 in range(HEADS_PER_CORE):
            for n2 in range(2):
                nc.tensor.matmul(
                    zz[:, n2, :],
                    OTs[:, h4, qb * 128 : (qb + 1) * 128],
                    wo_sb[:, h4, n2 * TCOL : (n2 + 1) * TCOL],
                    start=(h4 == 0),
                    stop=(h4 == HEADS_PER_CORE - 1),
                )
        zt = zp.tile([128, D], BF16)
        for n2 in range(2):
            nc.vector.tensor_copy(zt[:, n2 * TCOL : (n2 + 1) * TCOL], zz[:, n2, :])
        if not (phases & 8) or qb < 4:
            nc.sync.dma_start(
                out=partial[qb * 128 : (qb + 1) * 128, :], in_=zt[:]
            )

    if phases & 2:
        with (
            tc.tile_pool(name="ps_S", bufs=2, space="PSUM") as psS,
            tc.tile_pool(name="ps_O", bufs=1, space="PSUM") as psO,
            tc.tile_pool(name="ps_Z", bufs=1, space="PSUM") as psZ,
            tc.tile_pool(name="PT", bufs=4) as PTp,
            tc.tile_pool(name="zsb", bufs=3) as zp,
            tc.tile_pool(name="oU", bufs=3) as oUp,
            tc.tile_pool(name="rb", bufs=3) as rbp,
            tc.tile_pool(name="dn", bufs=3) as dnp,
        ):
            # Deferred work queues. Normalization of head h is emitted a few
            # blocks into head h+1's pipeline so its reciprocal chain never
            # blocks the in-order PE queue; phase-3 blocks of half 0 are
            # interleaved into half 1's attention to keep the PE at full
            # p-state during the activation-paced softmax.
            deferred_recip = []
            deferred_scale = []

            def make_recip(h, hh, oU):
                def run():
                    dnf = dnp.tile([1, 2, TCOL], F32, name=f"dnf{h}",
                                   tag=f"dnf{hh}")
                    if phases & 16:
                        nc.vector.memset(dnf[:], 1.0)  # timing-only
                    else:
                        nc.vector.reciprocal(dnf[:], oU[64:65, :, :])
                    return dnf
                return run

            def make_scale(qh, h, dnf_box, oU):
                def run():
                    dnf = dnf_box[0]
                    rbps = psZ.tile([64, 2, TCOL], F32,
                                    name="rbps", tag="zz")
                    for jq in range(2):
                        nc.tensor.matmul(
                            rbps[:, jq, :],
                            st["ones64f"][:],
                            dnf[:, jq, :],
                            start=True,
                            stop=True,
                        )
                    for jq in range(2):
                        nc.vector.tensor_mul(
                            OTs[:, h,
                                qh * 1024 + jq * TCOL : qh * 1024 + (jq + 1) * TCOL],
                            oU[0:64, jq, :],
                            rbps[:, jq, :],
                        )
                return run

            for qh in range(2):
                pending_qb = list(range(8)) if (qh == 1 and phases & 4) else []
                for t2 in range(2):
                    for hh in range(2):
                        h = 2 * t2 + hh
                        pO2 = psO.tile([65, 2, TCOL], F32, name=f"pO{h}",
                                       tag="pO")
                        # PV for block kb is emitted after the scores for
                        # kb+1, so the PE never waits on the exp of the
                        # block it just scored.
                        pts = [None] * NKB

                        def emit_pv(kb, pO2=pO2, pts=pts, h=h):
                            for jq in range(2):
                                nc.tensor.matmul(
                                    pO2[:, jq, :],
                                    VA[:, kb, h, :],
                                    pts[kb][:, jq * TCOL : (jq + 1) * TCOL],
                                    start=(kb == 0),
                                    stop=(kb == NKB - 1),
                                )

                        for kb in range(NKB):
                            ps = psS.tile([128, 1024], F32, name="ps")
                            for jq in range(2):
                                qoff = qh * 1024 + jq * TCOL
                                nc.tensor.matmul(
                                    ps[:, jq * TCOL : (jq + 1) * TCOL],
                                    KT[hh * 64 : hh * 64 + 64, t2,
                                       kb * 128 : (kb + 1) * 128],
                                    QT[hh * 64 : hh * 64 + 64, t2,
                                       qoff : qoff + TCOL],
                                    start=True,
                                    stop=True,
                                )
                            pt = PTp.tile([128, 1024], BF16, name="pt")
                            nc.scalar.activation(
                                pt[:], ps[:], mybir.ActivationFunctionType.Exp
                            )
                            pts[kb] = pt
                            if kb > 0:
                                emit_pv(kb - 1)
                            if kb == 2 and deferred_recip:
                                fn, box = deferred_recip.pop(0)
                                box[0] = fn()
                            first_head = qh == 1 and t2 == 0 and hh == 0
                            if pending_qb and (
                                kb == 14 or (not first_head and kb in (5, 10))
                            ):
                                # kb==14 > the kb==13 scale of the previous
                                # half's last head: OTs is complete before
                                # any output-projection block reads it.
                                phase3_qb(pending_qb.pop(0), psZ, zp)
                            if kb == 13 and deferred_scale:
                                deferred_scale.pop(0)()
                        emit_pv(NKB - 1)
                        # Drain PSUM -> SBUF immediately (frees pO for the
                        # next head); the normalize runs later, split so the
                        # slow reciprocal never gates a PSUM slot reuse.
                        oU = oUp.tile([65, 2, TCOL], F32, name=f"oU{h}",
                                      tag=f"oU{h % 3}")
                        nc.vector.tensor_copy(oU[:], pO2[0:65, :, :])
                        box = [None]
                        deferred_recip.append((make_recip(h, hh, oU), box))
                        deferred_scale.append(make_scale(qh, h, box, oU))
            # Flush the last head's normalize, then the phase-3 tail for
            # half 1 (half 0 was interleaved above).
            while deferred_recip:
                fn, box = deferred_recip.pop(0)
                box[0] = fn()
            while deferred_scale:
                deferred_scale.pop(0)()
            if phases & 4:
                for qb in range(8, 16):
                    phase3_qb(qb, psZ, zp)
    elif phases & 4:
        # Phase-3-only timing config.
        with (
            tc.tile_pool(name="ps_Zt", bufs=2, space="PSUM") as psZt,
            tc.tile_pool(name="zsbt", bufs=3) as zpt,
        ):
            for qb in range(16):
                phase3_qb(qb, psZt, zpt)




def build_attention_kernel(tc, es, ins, out_ext, loop_n=1, rs_n=1, phases=7):
    nc = tc.nc
    wq, wk, wv, wo = ins["wq"], ins["wk"], ins["wv"], ins["wo"]
    bq, bk = ins["bq"], ins["bk"]

    wpool = es.enter_context(tc.tile_pool(name="wpool", bufs=1))
    big = es.enter_context(tc.tile_pool(name="big", bufs=1))
    dram = es.enter_context(tc.tile_pool(name="dram", bufs=1, space="DRAM"))
    dram2 = es.enter_context(tc.tile_pool(name="dram2", bufs=2, space="DRAM"))

    # Weights into SBUF.
    wq_sb = wpool.tile([128, NIC, DG], BF16)
    wk_sb = wpool.tile([128, NIC, DG], BF16)
    wv_sb = wpool.tile([128, NIC, DG], BF16)
    nc.sync.dma_start(out=wq_sb[:], in_=wq.rearrange("(c p) d -> p c d", p=128))
    nc.sync.dma_start(out=wk_sb[:], in_=wk.rearrange("(c p) d -> p c d", p=128))
    nc.sync.dma_start(out=wv_sb[:], in_=wv.rearrange("(c p) d -> p c d", p=128))
    wo_sb = wpool.tile([64, HEADS_PER_CORE, D], BF16)
    nc.sync.dma_start(out=wo_sb[:], in_=wo.rearrange("(h p) n -> p h n", p=64))
    bq_sb = wpool.tile([128, 2, 1], F32)
    bk_sb = wpool.tile([128, 2, 1], F32)
    nc.sync.dma_start(out=bq_sb[:], in_=bq.rearrange("(c p) x -> p c x", p=128))
    nc.sync.dma_start(out=bk_sb[:], in_=bk.rearrange("(c p) x -> p c x", p=128))

    ident = wpool.tile([128, 128], BF16)
    make_identity(nc, ident[:])
    ones64 = wpool.tile([1, 64], BF16)
    nc.vector.memset(ones64[:], 1.0)
    ones64f = wpool.tile([1, 64], F32)
    nc.vector.memset(ones64f[:], 1.0)

    # Persistent activations.
    QT = big.tile([128, 2, S], BF16)  # [dim%128, dimblock, tok] = (q @ Wq).T
    KT = big.tile([128, 2, S], BF16)
    VA = big.tile([128, NKB, HEADS_PER_CORE, DH + 1], BF16)  # V + ones col
    OTs = big.tile([64, HEADS_PER_CORE, S], BF16)  # normalized O^T per head
    nc.vector.memset(VA[:, :, :, DH : DH + 1], 1.0)

    partial = dram.tile([S, D], BF16)
    rs_out = dram.tile([OUT_ROWS, D], BF16)

    st = dict(rs_out=rs_out,
        wq_sb=wq_sb, wk_sb=wk_sb, wv_sb=wv_sb, wo_sb=wo_sb,
        bq_sb=bq_sb, bk_sb=bk_sb, QT=QT, KT=KT, VA=VA, OTs=OTs,
        partial=partial, dram2=dram2, ident=ident, out_ext=out_ext,
        ones64=ones64, ones64f=ones64f,
    )

    if loop_n > 1:
        with tc.For_i(0, loop_n, 1):
            build_compute(tc, ins, st, phases=phases)
    else:
        build_compute(tc, ins, st, phases=phases)

    # ---- Single bf16 ReduceScatter over the whole partial buffer ----
    # (collectives cannot sit inside a hardware loop; rank g of each
    # batch group gets contiguous rows [512g : 512g+512])
    if rs_n == 0:
        # Timing-only fallback: wrong values, right shape.
        nc.sync.dma_start(out=out_ext, in_=partial[0:OUT_ROWS, :])
        return
    for _ in range(rs_n):
        nc.gpsimd.collective_compute(
            "ReduceScatter",
            mybir.AluOpType.add,
            replica_groups=[[0, 1, 2, 3], [4, 5, 6, 7]],
            ins=[partial[:].opt()],
            outs=[rs_out[:].opt()],
        )
    nc.sync.dma_start(out=out_ext, in_=rs_out[:])


def build_nc(loop_n=1, rs_n=1, phases=7):
    nc = bacc.Bacc(
        "TRN2", target_bir_lowering=False, debug=False, num_devices=NCORES
    )
    ins = {}
    for nm in ("q", "k", "v"):
        ins[nm] = nc.dram_tensor(nm, [S, D], BF16, kind="ExternalInput").ap()
    for nm in ("wq", "wk", "wv"):
        ins[nm] = nc.dram_tensor(nm, [D, DG], BF16, kind="ExternalInput").ap()
    ins["wo"] = nc.dram_tensor("wo", [DG, D], BF16, kind="ExternalInput").ap()
    ins["bq"] = nc.dram_tensor("bq", [DG, 1], F32, kind="ExternalInput").ap()
    ins["bk"] = nc.dram_tensor("bk", [DG, 1], F32, kind="ExternalInput").ap()
    out_ext = nc.dram_tensor("out", [OUT_ROWS, D], BF16, kind="ExternalOutput").ap()

    from contextlib import ExitStack

    with tile.TileContext(nc) as tc:
        with ExitStack() as es:
            build_attention_kernel(tc, es, ins, out_ext, loop_n=loop_n, rs_n=rs_n, phases=phases)
    nc.compile()
    return nc


def make_in_maps(q, k, v, Wq, bq, Wk, bk, Wv, bv, Wo, bo):
    """Host-side sharding. Attention scale (1/sqrt(64)) is folded into Wq.
    Matmul-path tensors are converted to bf16 on the host."""
    import ml_dtypes

    bf16 = ml_dtypes.bfloat16
    scale = DH**-0.5
    qb16 = [np.ascontiguousarray(q[b]).astype(bf16) for b in range(B)]
    kb16 = [np.ascontiguousarray(k[b]).astype(bf16) for b in range(B)]
    vb16 = [np.ascontiguousarray(v[b]).astype(bf16) for b in range(B)]
    in_maps = []
    for c in range(NCORES):
        b, g = c // 4, c % 4
        cols = slice(g * DG, (g + 1) * DG)
        in_maps.append(
            {
                "q": qb16[b],
                "k": kb16[b],
                "v": vb16[b],
                "wq": np.ascontiguousarray(Wq[:, cols] * scale).astype(bf16),
                "wk": np.ascontiguousarray(Wk[:, cols]).astype(bf16),
                "wv": np.ascontiguousarray(Wv[:, cols]).astype(bf16),
                "wo": np.ascontiguousarray(Wo[cols, :]).astype(bf16),
                "bq": np.ascontiguousarray(
                    (bq[cols] * scale).reshape(DG, 1), dtype=np.float32
                ),
                "bk": np.ascontiguousarray(
                    bk[cols].reshape(DG, 1), dtype=np.float32
                ),
            }
        )
    return in_maps


def assemble_output(results, bv, bo, Wo):
    out = np.empty((B, S, D), np.float32)
    for c in range(NCORES):
        b, g = c // 4, c % 4
        r = np.asarray(results[c]["out"], dtype=np.float32)
        out[b, g * OUT_ROWS : (g + 1) * OUT_ROWS, :] = r
    # bv's contribution commutes through softmax-normalized attention and the
    # output projection as a constant row offset; bo is a plain offset.
    bo_eff = np.asarray(bo, np.float64) + np.asarray(bv, np.float64) @ np.asarray(
        Wo, np.float64
    )
    if np.any(bo_eff):
        out += bo_eff[None, None, :].astype(np.float32)
    return out


_NC_CACHE = None


def kernel(q, k, v, Wq, bq, Wk, bk, Wv, bv, Wo, bo):
    global _NC_CACHE
    from concourse.bass_utils import run_bass_kernel_spmd

    args = [
        np.asarray(x, np.float32) for x in (q, k, v, Wq, bq, Wk, bk, Wv, bv, Wo, bo)
    ]
    q, k, v, Wq, bq, Wk, bk, Wv, bv, Wo, bo = args
    if _NC_CACHE is None:
        _NC_CACHE = build_nc()
    nc = _NC_CACHE
    in_maps = make_in_maps(q, k, v, Wq, bq, Wk, bk, Wv, bv, Wo, bo)
    res = run_bass_kernel_spmd(nc, in_maps, core_ids=list(range(NCORES)))
    return assemble_output(res.results, bv, bo, Wo)


# revision 3
# speedup vs baseline: 1.0046x; 1.0046x over previous
"""Cross-attention kernel for Trainium2, sharded across 8 NeuronCores.

Sharding: data-parallel over batch (B=2) x tensor-parallel over head groups
(16 heads -> 4 groups of 4). Core c handles batch c//4, head group c%4.
Each core projects with its 256-wide column shard of Wq/Wk/Wv, runs attention
for its 4 heads, applies its 256-row shard of Wo, and a ReduceScatter over
each batch group of 4 cores sums the partial outputs and hands each core its
512-row slice of the final output.

v2 layout: query-half-outer attention loop; the output projection and the
(bf16) ReduceScatter chunk for a half are issued as soon as that half's
attention finishes, so the collective overlaps the other half's compute.
PSUM attention accumulators are drained to SBUF by the (otherwise idle)
Pool engine so softmax normalization never blocks the tensor engine.

Matmul-path data is bf16 (converted host-side); softmax statistics and PSUM
accumulation stay fp32. The attention scale 1/sqrt(64) is folded into Wq on
the host.
"""

import sys

sys.path.insert(0, "/opt/trn_rl_repo")

import numpy as np

import concourse.bass as bass
import concourse.mybir as mybir
import concourse.tile as tile
from concourse import bacc
from concourse.masks import make_identity

F32 = mybir.dt.float32
BF16 = mybir.dt.bfloat16

B = 2
S = 2048  # both Sq and Sk
D = 1024
NCORES = 8
HEADS_PER_CORE = 4
DH = 64
DG = HEADS_PER_CORE * DH  # 256: per-core projection width
TCOL = 512  # token column width for projections / attention moving dim
NTCOL = S // TCOL  # 4
NIC = D // 128  # 8 input-dim chunks
NKB = S // 128  # 16 key blocks
NQB = S // 128  # 16 query blocks
OUT_ROWS = S // 4  # 512 rows of final output per core (ReduceScatter shard)


def build_compute(tc, ins, st, phases=7):
    """One full pass: projections, attention, output projection, and the
    chunked ReduceScatter. May sit inside a timing repeat loop (all state
    tiles are rewritten every pass).

    Bits 0/1/2 enable phases 1/2/3. Timing-only modifier bits (results
    become wrong): bit3 (8) = phase 3 skips the DRAM write; bit4 (16) =
    phase 2 skips the denominator broadcast round-trip."""
    nc = tc.nc
    q, k, v = ins["q"], ins["k"], ins["v"]
    wq_sb, wk_sb, wv_sb, wo_sb = st["wq_sb"], st["wk_sb"], st["wv_sb"], st["wo_sb"]
    bq_sb, bk_sb = st["bq_sb"], st["bk_sb"]
    QT, KT, VA, OTs = st["QT"], st["KT"], st["VA"], st["OTs"]
    partial, dram2 = st["partial"], st["dram2"]
    out_ext = st["out_ext"]

    # ---- Phase 1: PE-transpose inputs + projections ----
    ident = st["ident"]
    if phases & 1:
        with (
            tc.tile_pool(name="nat", bufs=3) as natp,
            tc.tile_pool(name="tT", bufs=3) as tTp,
            tc.tile_pool(name="ps_t", bufs=2, space="PSUM") as ps_t,
            tc.tile_pool(name="ps_p", bufs=4, space="PSUM") as ps_p,
        ):
            for which, src in (("q", q), ("k", k), ("v", v)):
                for tcol in range(NTCOL):
                    nat = natp.tile([128, 4, D], BF16)
                    rows = src[tcol * TCOL : (tcol + 1) * TCOL, :]
                    nc.sync.dma_start(
                        out=nat[:], in_=rows.rearrange("(b p) d -> p b d", p=128)
                    )
                    tT = tTp.tile([128, NIC, TCOL], BF16)
                    for ic in range(NIC):
                        pst = ps_t.tile([128, TCOL], BF16)
                        for tb in range(4):
                            nc.tensor.transpose(
                                pst[:, tb * 128 : (tb + 1) * 128],
                                nat[:, tb, ic * 128 : (ic + 1) * 128],
                                ident[:],
                            )
                        nc.vector.tensor_copy(tT[:, ic, :], pst[:])
                    if which in ("q", "k"):
                        dstT = QT if which == "q" else KT
                        bias = bq_sb if which == "q" else bk_sb
                        w_sb = wq_sb if which == "q" else wk_sb
                        for db in range(2):
                            pp = ps_p.tile([128, TCOL], F32)
                            for ic in range(NIC):
                                nc.tensor.matmul(
                                    pp[:],
                                    w_sb[:, ic, db * 128 : (db + 1) * 128],
                                    tT[:, ic, :],
                                    start=(ic == 0),
                                    stop=(ic == NIC - 1),
                                )
                            nc.vector.tensor_scalar_add(
                                dstT[:, db, tcol * TCOL : (tcol + 1) * TCOL],
                                pp[:],
                                bias[:, db, :],
                            )
                    else:
                        for tb in range(4):
                            pp = ps_p.tile([128, TCOL], F32)
                            for ic in range(NIC):
                                nc.tensor.matmul(
                                    pp[:, 0:DG],
                                    tT[:, ic, tb * 128 : (tb + 1) * 128],
                                    wv_sb[:, ic, :],
                                    start=(ic == 0),
                                    stop=(ic == NIC - 1),
                                )
                            kb = tcol * 4 + tb
                            nc.vector.tensor_copy(
                                VA[:, kb, :, 0:DH],
                                pp[:, 0:DG].rearrange("p (h d) -> p h d", d=DH),
                            )

    # ---- Phases 2+3 per query half, with the RS chunk issued per half ----
    def phase3_qb(qb, psZ, zp):
        """Output projection for one 128-query block (8 matmuls + drain)."""
        zz = psZ.tile([128, 2, TCOL], F32, name=f"zz{qb}", tag="zz")
        for h4 in range(HEADS_PER_CORE):
            for n2 in range(2):
                nc.tensor.matmul(
                    zz[:, n2, :],
                    OTs[:, h4, qb * 128 : (qb + 1) * 128],
                    wo_sb[:, h4, n2 * TCOL : (n2 + 1) * TCOL],
                    start=(h4 == 0),
                    stop=(h4 == HEADS_PER_CORE - 1),
                )
        zt = zp.tile([128, D], BF16)
        for n2 in range(2):
            nc.vector.tensor_copy(zt[:, n2 * TCOL : (n2 + 1) * TCOL], zz[:, n2, :])
        if not (phases & 8) or qb < 4:
            nc.sync.dma_start(
                out=partial[qb * 128 : (qb + 1) * 128, :], in_=zt[:]
            )

    if phases & 2:
        with (
            tc.tile_pool(name="ps_S", bufs=2, space="PSUM") as psS,
            tc.tile_pool(name="ps_O", bufs=1, space="PSUM") as psO,
            tc.tile_pool(name="ps_Z", bufs=1, space="PSUM") as psZ,
            tc.tile_pool(name="PT", bufs=4) as PTp,
            tc.tile_pool(name="zsb", bufs=3) as zp,
            tc.tile_pool(name="oU", bufs=3) as oUp,
            tc.tile_pool(name="rb", bufs=3) as rbp,
            tc.tile_pool(name="dn", bufs=3) as dnp,
        ):
            # Inline normalization: non-last heads use the DRAM round-trip
            # broadcast (fully hidden under the next head's ~17us pipeline);
            # the last head of each half (which gates phase 3) uses a K=1
            # ones-matmul broadcast into a then-idle scores slot.
            def normalize(qh, h, hh, last_head, oU):
                if phases & 16:
                    rb = rbp.tile([64, 1024], F32, name=f"rb{hh}",
                                  tag=f"rb{hh}")
                    nc.vector.memset(rb[:], 1.0)  # timing-only stand-in
                    rbv = rb[:]
                elif not last_head:
                    dn = dnp.tile([1, 2, TCOL], F32, name=f"dn{h}",
                                  tag=f"dn{hh}")
                    nc.vector.reciprocal(dn[:], oU[64:65, :, :])
                    rb = rbp.tile([64, 1024], F32, name=f"rb{hh}",
                                  tag=f"rb{hh}")
                    scr = dram2.tile([1, 1024], F32, name="scr", tag="scr")
                    nc.sync.dma_start(out=scr[:], in_=dn[:])
                    scr_ap = scr[:]
                    bcast = bass.AP(
                        tensor=scr_ap.tensor,
                        offset=scr_ap.offset,
                        ap=[[0, 64], [1, 1024]],
                    )
                    nc.sync.dma_start(out=rb[:], in_=bcast)
                    rbv = rb[:]
                else:
                    dnb = dnp.tile([1, 2, TCOL], BF16, name=f"dnb{h}",
                                   tag=f"dnb{hh}")
                    with nc.allow_low_precision(
                        reason="bf16 softmax denom: 0.4% scale noise ok"
                    ):
                        nc.vector.reciprocal(dnb[:], oU[64:65, :, :])
                    rbps = psS.tile([64, 2, TCOL], F32,
                                    name="rbps", tag="ps")
                    for jq in range(2):
                        nc.tensor.matmul(
                            rbps[:, jq, :],
                            st["ones64"][:],
                            dnb[:, jq, :],
                            start=True,
                            stop=True,
                        )
                    rbv = rbps.rearrange("p a b -> p (a b)")
                for jq in range(2):
                    nc.vector.tensor_mul(
                        OTs[:, h,
                            qh * 1024 + jq * TCOL : qh * 1024 + (jq + 1) * TCOL],
                        oU[0:64, jq, :],
                        rbv[:, jq * TCOL : (jq + 1) * TCOL],
                    )

            for qh in range(2):
                pending_qb = list(range(8)) if (qh == 1 and phases & 4) else []
                for t2 in range(2):
                    for hh in range(2):
                        h = 2 * t2 + hh
                        last_head = t2 == 1 and hh == 1
                        pO2 = psO.tile([65, 2, TCOL], F32, name=f"pO{h}",
                                       tag="pO")
                        for kb in range(NKB):
                            ps = psS.tile([128, 1024], F32, name="ps")
                            for jq in range(2):
                                qoff = qh * 1024 + jq * TCOL
                                nc.tensor.matmul(
                                    ps[:, jq * TCOL : (jq + 1) * TCOL],
                                    KT[hh * 64 : hh * 64 + 64, t2,
                                       kb * 128 : (kb + 1) * 128],
                                    QT[hh * 64 : hh * 64 + 64, t2,
                                       qoff : qoff + TCOL],
                                    start=True,
                                    stop=True,
                                )
                            pt = PTp.tile([128, 1024], BF16, name="pt")
                            nc.scalar.activation(
                                pt[:], ps[:], mybir.ActivationFunctionType.Exp
                            )
                            for jq in range(2):
                                nc.tensor.matmul(
                                    pO2[:, jq, :],
                                    VA[:, kb, h, :],
                                    pt[:, jq * TCOL : (jq + 1) * TCOL],
                                    start=(kb == 0),
                                    stop=(kb == NKB - 1),
                                )
                            if pending_qb and kb % 8 == 7:
                                phase3_qb(pending_qb.pop(0), psZ, zp)
                        # Drain PSUM -> SBUF immediately (frees pO for the
                        # next head), then normalize from SBUF.
                        oU = oUp.tile([65, 2, TCOL], F32, name=f"oU{h}",
                                      tag=f"oU{h % 3}")
                        nc.vector.tensor_copy(oU[:], pO2[0:65, :, :])
                        normalize(qh, h, hh, last_head, oU)
            if phases & 4:
                for qb in range(8, 16):
                    phase3_qb(qb, psZ, zp)
    elif phases & 4:
        # Phase-3-only timing config.
        with (
            tc.tile_pool(name="ps_Zt", bufs=2, space="PSUM") as psZt,
            tc.tile_pool(name="zsbt", bufs=3) as zpt,
        ):
            for qb in range(16):
                phase3_qb(qb, psZt, zpt)




def build_attention_kernel(tc, es, ins, out_ext, loop_n=1, rs_n=1, phases=7):
    nc = tc.nc
    wq, wk, wv, wo = ins["wq"], ins["wk"], ins["wv"], ins["wo"]
    bq, bk = ins["bq"], ins["bk"]

    wpool = es.enter_context(tc.tile_pool(name="wpool", bufs=1))
    big = es.enter_context(tc.tile_pool(name="big", bufs=1))
    dram = es.enter_context(tc.tile_pool(name="dram", bufs=1, space="DRAM"))
    dram2 = es.enter_context(tc.tile_pool(name="dram2", bufs=2, space="DRAM"))

    # Weights into SBUF.
    wq_sb = wpool.tile([128, NIC, DG], BF16)
    wk_sb = wpool.tile([128, NIC, DG], BF16)
    wv_sb = wpool.tile([128, NIC, DG], BF16)
    nc.sync.dma_start(out=wq_sb[:], in_=wq.rearrange("(c p) d -> p c d", p=128))
    nc.sync.dma_start(out=wk_sb[:], in_=wk.rearrange("(c p) d -> p c d", p=128))
    nc.sync.dma_start(out=wv_sb[:], in_=wv.rearrange("(c p) d -> p c d", p=128))
    wo_sb = wpool.tile([64, HEADS_PER_CORE, D], BF16)
    nc.sync.dma_start(out=wo_sb[:], in_=wo.rearrange("(h p) n -> p h n", p=64))
    bq_sb = wpool.tile([128, 2, 1], F32)
    bk_sb = wpool.tile([128, 2, 1], F32)
    nc.sync.dma_start(out=bq_sb[:], in_=bq.rearrange("(c p) x -> p c x", p=128))
    nc.sync.dma_start(out=bk_sb[:], in_=bk.rearrange("(c p) x -> p c x", p=128))

    ident = wpool.tile([128, 128], BF16)
    make_identity(nc, ident[:])
    ones64 = wpool.tile([1, 64], BF16)
    nc.vector.memset(ones64[:], 1.0)
    ones64f = wpool.tile([1, 64], F32)
    nc.vector.memset(ones64f[:], 1.0)

    # Persistent activations.
    QT = big.tile([128, 2, S], BF16)  # [dim%128, dimblock, tok] = (q @ Wq).T
    KT = big.tile([128, 2, S], BF16)
    VA = big.tile([128, NKB, HEADS_PER_CORE, DH + 1], BF16)  # V + ones col
    OTs = big.tile([64, HEADS_PER_CORE, S], BF16)  # normalized O^T per head
    nc.vector.memset(VA[:, :, :, DH : DH + 1], 1.0)

    partial = dram.tile([S, D], BF16)
    rs_out = dram.tile([OUT_ROWS, D], BF16)

    st = dict(rs_out=rs_out,
        wq_sb=wq_sb, wk_sb=wk_sb, wv_sb=wv_sb, wo_sb=wo_sb,
        bq_sb=bq_sb, bk_sb=bk_sb, QT=QT, KT=KT, VA=VA, OTs=OTs,
        partial=partial, dram2=dram2, ident=ident, out_ext=out_ext,
        ones64=ones64, ones64f=ones64f,
    )

    if loop_n > 1:
        with tc.For_i(0, loop_n, 1):
            build_compute(tc, ins, st, phases=phases)
    else:
        build_compute(tc, ins, st, phases=phases)

    # ---- Single bf16 ReduceScatter over the whole partial buffer ----
    # (collectives cannot sit inside a hardware loop; rank g of each
    # batch group gets contiguous rows [512g : 512g+512])
    if rs_n == 0:
        # Timing-only fallback: wrong values, right shape.
        nc.sync.dma_start(out=out_ext, in_=partial[0:OUT_ROWS, :])
        return
    for _ in range(rs_n):
        nc.gpsimd.collective_compute(
            "ReduceScatter",
            mybir.AluOpType.add,
            replica_groups=[[0, 1, 2, 3], [4, 5, 6, 7]],
            ins=[partial[:].opt()],
            outs=[rs_out[:].opt()],
        )
    nc.sync.dma_start(out=out_ext, in_=rs_out[:])


def build_nc(loop_n=1, rs_n=1, phases=7):
    nc = bacc.Bacc(
        "TRN2", target_bir_lowering=False, debug=False, num_devices=NCORES
    )
    ins = {}
    for nm in ("q", "k", "v"):
        ins[nm] = nc.dram_tensor(nm, [S, D], BF16, kind="ExternalInput").ap()
    for nm in ("wq", "wk", "wv"):
        ins[nm] = nc.dram_tensor(nm, [D, DG], BF16, kind="ExternalInput").ap()
    ins["wo"] = nc.dram_tensor("wo", [DG, D], BF16, kind="ExternalInput").ap()
    ins["bq"] = nc.dram_tensor("bq", [DG, 1], F32, kind="ExternalInput").ap()
    ins["bk"] = nc.dram_tensor("bk", [DG, 1], F32, kind="ExternalInput").ap()
    out_ext = nc.dram_tensor("out", [OUT_ROWS, D], BF16, kind="ExternalOutput").ap()

    from contextlib import ExitStack

    with tile.TileContext(nc) as tc:
        with ExitStack() as es:
            build_attention_kernel(tc, es, ins, out_ext, loop_n=loop_n, rs_n=rs_n, phases=phases)
    nc.compile()
    return nc


def make_in_maps(q, k, v, Wq, bq, Wk, bk, Wv, bv, Wo, bo):
    """Host-side sharding. Attention scale (1/sqrt(64)) is folded into Wq.
    Matmul-path tensors are converted to bf16 on the host."""
    import ml_dtypes

    bf16 = ml_dtypes.bfloat16
    scale = DH**-0.5
    qb16 = [np.ascontiguousarray(q[b]).astype(bf16) for b in range(B)]
    kb16 = [np.ascontiguousarray(k[b]).astype(bf16) for b in range(B)]
    vb16 = [np.ascontiguousarray(v[b]).astype(bf16) for b in range(B)]
    in_maps = []
    for c in range(NCORES):
        b, g = c // 4, c % 4
        cols = slice(g * DG, (g + 1) * DG)
        in_maps.append(
            {
                "q": qb16[b],
                "k": kb16[b],
                "v": vb16[b],
                "wq": np.ascontiguousarray(Wq[:, cols] * scale).astype(bf16),
                "wk": np.ascontiguousarray(Wk[:, cols]).astype(bf16),
                "wv": np.ascontiguousarray(Wv[:, cols]).astype(bf16),
                "wo": np.ascontiguousarray(Wo[cols, :]).astype(bf16),
                "bq": np.ascontiguousarray(
                    (bq[cols] * scale).reshape(DG, 1), dtype=np.float32
                ),
                "bk": np.ascontiguousarray(
                    bk[cols].reshape(DG, 1), dtype=np.float32
                ),
            }
        )
    return in_maps


def assemble_output(results, bv, bo, Wo):
    out = np.empty((B, S, D), np.float32)
    for c in range(NCORES):
        b, g = c // 4, c % 4
        r = np.asarray(results[c]["out"], dtype=np.float32)
        out[b, g * OUT_ROWS : (g + 1) * OUT_ROWS, :] = r
    # bv's contribution commutes through softmax-normalized attention and the
    # output projection as a constant row offset; bo is a plain offset.
    bo_eff = np.asarray(bo, np.float64) + np.asarray(bv, np.float64) @ np.asarray(
        Wo, np.float64
    )
    if np.any(bo_eff):
        out += bo_eff[None, None, :].astype(np.float32)
    return out


_NC_CACHE = None


def kernel(q, k, v, Wq, bq, Wk, bk, Wv, bv, Wo, bo):
    global _NC_CACHE
    from concourse.bass_utils import run_bass_kernel_spmd

    args = [
        np.asarray(x, np.float32) for x in (q, k, v, Wq, bq, Wk, bk, Wv, bv, Wo, bo)
    ]
    q, k, v, Wq, bq, Wk, bk, Wv, bv, Wo, bo = args
    if _NC_CACHE is None:
        _NC_CACHE = build_nc()
    nc = _NC_CACHE
    in_maps = make_in_maps(q, k, v, Wq, bq, Wk, bk, Wv, bv, Wo, bo)
    res = run_bass_kernel_spmd(nc, in_maps, core_ids=list(range(NCORES)))
    return assemble_output(res.results, bv, bo, Wo)
